# revision 50
# baseline (speedup 1.0000x reference)
"""Trainium2 Bass kernel for nn_BERT4GCN_53884659695997.

Mathematical reduction
----------------------
In the reference, ``feature`` is reassigned to ``LN(guidance)`` at the top of
every loop iteration, so the GCN block's output is never consumed; only the
last BERT layer's branch (index 3 -> hidden_states layer 12, which skips the
GCN block) reaches the output:

    t[b]      = LN(relu(hs[12,b][ts[b]] @ guid_W[3] + guid_b[3])) * ln_g + ln_b
    logits[b] = ((t[b] * m[b,:,None]).sum(0) / m[b].sum(0)) @ cls_W + cls_b

(verified numerically against the jax reference to ~7e-7 rel err).

Row gathers commute with the row-wise ops (matmul-by-row / relu / LN), so the
gather+mask folds into per-source-row weights w[r] = sum_i m[i]*[ts[i]==r].
Only rows with w[r] != 0 can reach the output (at most |unique(ts[m>0])| ~ 51
per sample), so each sample's work is compacted to KC=64 rows, and two
samples are packed per 128-partition tile (pair P = samples 2P, 2P+1), which
halves the per-sample instruction count.  The compact row lists are emitted
host-side (pure index bookkeeping); ONE fused SWDGE gather DMA per iteration
pulls all 8 samples' rows out of the flattened [B*L, D] view (a dma_gather
launch costs ~2.5us on silicon and rows stream at ~111 GB/s, so batching the
launch and minimizing gathered bytes are what the hardware actually rewards).

The gathered rows land row-major [j, d]; the guidance matmul contracts over d,
so six 128-col PE transposes produce the [d, j] stationary operand.  The
guidance matmul streams guid_W as float32r (full rate for moving dims >= 256);
bias enters as a K=1 ones-row matmul.  The emission is software-pipelined in
five stages across three iterations of lag so the PE stream never waits on the
DVE/ACT stats or copy stages, and all activation functions (Relu / Copy /
Abs_reciprocal_sqrt) live in one act-table set so no LoadActFuncSet lands in
the steady-state loop.

LN folds into the reduction: with per-row stats (mu, rs) from one-pass
bn_stats/bn_aggr and w2 = w * rs,

    sum_r w[r]*(GR[r,:]-mu[r])*rs[r] = GR^T @ w2 - (mu . w2) * ones

mu is carried as a 601st column of the activation tile so the aspect matmul
emits the correction term alongside, and cls_W gains a 601st row equal to
-colsum(ln_g*cls_W), which applies the correction exactly.  ln_g / ln_b fold
into cls_W / cls_b host-side (exact fp32 linear algebra).

Sharding: data-parallel over batch B=64 -> 8 samples per core on 8 cores.
"""

import numpy as np
from contextlib import ExitStack

import concourse.bass as bass
import concourse.tile as tile
from concourse import bacc, mybir
from concourse.bass_utils import run_bass_kernel_spmd

F32 = mybir.dt.float32
F32R = mybir.dt.float32r
I16 = mybir.dt.int16
AX = mybir.AxisListType
ALU = mybir.AluOpType
ACTF = mybir.ActivationFunctionType

N_CORES = 8
B = 64
BC = B // N_CORES
L = 256
D = 768
H = 600
KC = 64         # compact row budget per sample (unique masked starts ~51)
NP = 4          # sample pairs per core: pair P = samples (2P, 2P+1) sharing 128 partitions
EPS = 1e-5
KT = D // 128   # 6 k-tiles
NCH = ((0, 344), (344, 600))           # guidance chunks, both >= 256 for f32r
# aspect/classifier chunks over the extended 601-wide feature (600 + mu col)
ACH = ((0, 128), (128, 256), (256, 384), (384, 512), (512, 601))
IDXW = KC // 16  # idx cols per sample (SWDGE wraps indices over 16 partitions)


def build_program(repeats: int = 1):
    nc = bacc.Bacc("TRN2", target_bir_lowering=False, debug=False,
                   num_devices=N_CORES, dynamic_dma_scratch_size=32768)

    dr = {}
    def din(name, shape, dt=F32):
        dr[name] = nc.dram_tensor(name, list(shape), dt, kind="ExternalInput").ap()
    din("hs", (BC, L, D))
    din("idx", (128, BC * KC // 16), I16)
    din("gw", (D, H))
    din("gbrow", (1, H))
    din("onesrow", (1, 128))
    din("eye", (128, 128))
    din("tscT", (L, BC))
    din("mT", (L, BC))
    din("mnat", (BC, L))
    din("iota", (128, 2 * KC))
    din("clsw", (640, 3))         # ln_g-folded cls_W + correction row, padded
    din("clsb", (BC, 3))          # ln_b@cls_W + cls_b, replicated rows
    out_ap = nc.dram_tensor("out", [BC, 3], F32, kind="ExternalOutput").ap()

    with tile.TileContext(nc) as tc, ExitStack() as ctx:
        cpool = ctx.enter_context(tc.tile_pool(name="consts", bufs=1))
        gpool = ctx.enter_context(tc.tile_pool(name="hsc", bufs=3))
        tpool = ctx.enter_context(tc.tile_pool(name="hst", bufs=3))
        grpool = ctx.enter_context(tc.tile_pool(name="gr2", bufs=4))
        spool = ctx.enter_context(tc.tile_pool(name="small", bufs=2))
        stats = ctx.enter_context(tc.tile_pool(name="stats", bufs=1))
        pg_ps = ctx.enter_context(tc.tile_pool(name="pg", bufs=3, space="PSUM"))
        pgu_ps = ctx.enter_context(tc.tile_pool(name="pgu", bufs=3, space="PSUM"))
        sm_ps = ctx.enter_context(tc.tile_pool(name="sm", bufs=1, space="PSUM"))
        asp_ps = ctx.enter_context(tc.tile_pool(name="asp", bufs=1, space="PSUM"))

        # ---- constants (loaded once; idx/eye first so gathers start early) ----
        # NOTE: float32r must never touch a DMA on this backend (transfers
        # are lossy); f32r tiles are produced by on-chip engine copies only.
        IDX = cpool.tile([128, BC * KC // 16], I16, tag="idx")
        nc.sync.dma_start(IDX[:], dr["idx"][:])
        EYE = cpool.tile([128, 128], F32, tag="eye")
        nc.sync.dma_start(EYE[:], dr["eye"][:])
        GW0 = cpool.tile([128, KT, H], F32, tag="gw0")
        gw_r = dr["gw"].rearrange("(k p) n -> p k n", p=128)
        for kt in range(KT):   # split so early gathers interleave on the DMA engines
            nc.sync.dma_start(GW0[:, kt, :], gw_r[:, kt, :])
        GW = cpool.tile([128, KT, H], F32R, tag="gw")
        for kt in range(KT):   # spread the one-time cast over two engines
            if kt % 2 == 0:
                nc.vector.tensor_copy(GW[:, kt, :], GW0[:, kt, :])
            else:
                nc.scalar.copy(GW[:, kt, :], GW0[:, kt, :])
        GBROW0 = cpool.tile([1, H], F32, tag="gbrow0")
        nc.sync.dma_start(GBROW0[:], dr["gbrow"][:])
        GBROW = cpool.tile([1, H], F32R, tag="gbrow")
        nc.vector.tensor_copy(GBROW[:], GBROW0[:])
        ONESR0 = cpool.tile([1, 128], F32, tag="onesrow0")
        nc.sync.dma_start(ONESR0[:], dr["onesrow"][:])
        ONESR = cpool.tile([1, 128], F32R, tag="onesrow")
        nc.vector.tensor_copy(ONESR[:], ONESR0[:])
        IOTA = cpool.tile([128, 2 * KC], F32, tag="iota")
        nc.sync.dma_start(IOTA[:], dr["iota"][:])
        TSC = cpool.tile([128, 2, BC], F32, tag="tsc")
        nc.sync.dma_start(TSC[:], dr["tscT"].rearrange("(t p) s -> p t s", p=128))
        MT = cpool.tile([128, 2, BC], F32, tag="mt")
        nc.sync.dma_start(MT[:], dr["mT"].rearrange("(t p) s -> p t s", p=128))
        MN = cpool.tile([BC, L], F32, tag="mn")
        nc.sync.dma_start(MN[:], dr["mnat"][:])
        CLSW = cpool.tile([128, 5, 3], F32, tag="clsw")
        nc.sync.dma_start(CLSW[:], dr["clsw"].rearrange("(c p) n -> p c n", p=128))
        CLSB = cpool.tile([BC, 3], F32, tag="clsb")
        nc.sync.dma_start(CLSB[:], dr["clsb"][:])

        ONESC = cpool.tile([128, 1], F32, tag="onesc")
        nc.vector.memset(ONESC[:], 1.0)
        EPSB = stats.tile([128, 1], F32, tag="epsb")
        nc.vector.memset(EPSB[:], EPS)
        # dummy activation so the preamble exits with the same act-table set
        # the loop body uses -- keeps LoadActFuncSet out of the repeat loop
        DUM = stats.tile([1, 1], F32, tag="dum")
        nc.scalar.activation(DUM[:], EPSB[0:1, :], ACTF.Abs_reciprocal_sqrt)

        # 1/sum(m) per sample
        SM = stats.tile([BC, 1], F32, tag="sm")
        nc.vector.tensor_reduce(SM[:], MN[:], AX.X, ALU.add)
        RECIP = stats.tile([BC, 1], F32, tag="recip")
        nc.vector.reciprocal(RECIP[:], SM[:])

        def body():
            ASPT = asp_ps.tile([128, 5 * BC], F32, tag="aspt")
            # zero the full tile so untouched partitions (chunk 4 rows > 89)
            # contribute exact zeros to the classifier contraction
            nc.vector.memset(ASPT[:], 0.0)
            state = {}

            hst_state = {}

            # two half-gathers (256 rows each): pair 0/1 compute starts as
            # soon as half A lands while half B still streams, and the next
            # body's half A overlaps this body's tail -- a single fused
            # gather would serialize all compute behind the full transfer
            nidx = BC * KC // 2
            HSCA = gpool.tile([128, 2, D], F32, tag="hscfa")
            nc.gpsimd.dma_gather(
                HSCA[:], dr["hs"].rearrange("b l d -> (b l) d"),
                IDX[:, :nidx // 16], nidx, nidx, D)
            HSCB = gpool.tile([128, 2, D], F32, tag="hscfb")
            nc.gpsimd.dma_gather(
                HSCB[:], dr["hs"].rearrange("b l d -> (b l) d"),
                IDX[:, nidx // 16:], nidx, nidx, D)
            halves = {0: (HSCA, 0), 1: (HSCA, 1), 2: (HSCB, 0), 3: (HSCB, 1)}

            def front_a(p):
                """transpose pair p -> copy-to-SBUF (cast to f32r)."""
                TPa = pg_ps.tile([128, 384], F32, tag="pg")
                TPb = pg_ps.tile([128, 384], F32, tag="pg")
                HSC, ci = halves[p]
                for k in range(3):
                    nc.tensor.transpose(
                        TPa[:, k * 128:(k + 1) * 128],
                        HSC[:, ci, k * 128:(k + 1) * 128], EYE[:])
                for k in range(3):
                    nc.tensor.transpose(
                        TPb[:, k * 128:(k + 1) * 128],
                        HSC[:, ci, (k + 3) * 128:(k + 4) * 128], EYE[:])
                HST = tpool.tile([128, KT, 128], F32R, tag="hst")
                nc.vector.tensor_copy(HST[:, 0:3, :], TPa[:])
                nc.scalar.copy(HST[:, 3:6, :], TPb[:])
                hst_state[p] = HST

            def front_b(p):
                """guidance matmul -> relu -> one-pass LN stats (pair-wide)."""
                HST = hst_state.pop(p)
                GR2 = grpool.tile([128, 601], F32, tag="gr2")
                for ci, (nlo, nhi) in enumerate(NCH):
                    PG = pgu_ps.tile([128, nhi - nlo], F32, tag="pgu")
                    for kt in range(KT):
                        nc.tensor.matmul(
                            PG[:], HST[:, kt, :], GW[:, kt, nlo:nhi],
                            start=(kt == 0), stop=False)
                    nc.tensor.matmul(
                        PG[:], ONESR[:], GBROW[:, nlo:nhi], start=False, stop=True)
                    nc.scalar.activation(GR2[:, nlo:nhi], PG[:], ACTF.Relu)
                # 600 = 4 equal half-chunks of 150 -> bn_aggr pooling is exact
                BST = spool.tile([128, 12], F32, tag="bst")
                nc.vector.bn_stats(BST[:, 0:6], GR2[:, 0:300])
                nc.vector.bn_stats(BST[:, 6:12], GR2[:, 300:600])
                AGG = spool.tile([128, 2], F32, tag="agg")
                nc.vector.bn_aggr(AGG[:], BST[:])
                state[p] = (GR2, AGG)

            def back_stats(p):
                """rstd chain + mask-fused one-hots for both pair members."""
                GR2, AGG = state[p]
                RS = spool.tile([128, 1], F32, tag="rs")
                nc.scalar.activation(RS[:], AGG[:, 1:2], ACTF.Abs_reciprocal_sqrt,
                                     bias=EPSB[:])
                nc.vector.tensor_copy(GR2[:, 600:601], AGG[:, 0:1])
                SOHW = spool.tile([128, 2, 128], F32, tag="soh")
                for it in range(2):
                    for h in range(2):
                        sx = 2 * p + h
                        nc.vector.tensor_scalar(
                            SOHW[:, it, h * KC:(h + 1) * KC],
                            IOTA[:, :KC], TSC[:, it, sx:sx + 1],
                            MT[:, it, sx:sx + 1], ALU.is_equal, ALU.mult)
                state[p] = (GR2, RS, SOHW)

            def back_wg(p):
                """gather-weight matmuls (PE, early in the stream)."""
                GR2, RS, SOHW = state[p]
                WPS = sm_ps.tile([128, 1], F32, tag="sm")
                for it in range(2):
                    nc.tensor.matmul(
                        WPS[:], SOHW[:, it, :], ONESC[:],
                        start=(it == 0), stop=(it == 1))
                W2 = spool.tile([128, 1], F32, tag="w2")
                nc.vector.tensor_mul(W2[:], WPS[:], RS[:])
                state[p] = (GR2, W2)

            def back_asp(p):
                """aspect columns for both pair members (PE, end of stream)."""
                GR2, W2 = state.pop(p)
                for h in range(2):
                    sx = 2 * p + h
                    lo, hi = h * KC, (h + 1) * KC
                    for hc, (hlo, hhi) in enumerate(ACH):
                        nc.tensor.matmul(
                            ASPT[:hhi - hlo, hc * BC + sx:hc * BC + sx + 1],
                            GR2[lo:hi, hlo:hhi], W2[lo:hi, :])

            # software-pipelined emission with iteration lags so the PE
            # stream never stalls on the DVE/ACT copy or stats stages
            LAG = 3
            for i in range(NP + LAG):
                if i >= LAG:
                    back_wg(i - LAG)
                if i < NP:
                    front_a(i)
                if 2 <= i < NP + 2:
                    back_stats(i - 2)
                if 1 <= i <= NP:
                    front_b(i - 1)
                if i >= LAG:
                    back_asp(i - LAG)

            # -------- classifier --------
            ASB = stats.tile([128, 5, BC], F32, tag="asb")
            nc.scalar.copy(ASB[:], ASPT[:])
            LG = sm_ps.tile([BC, 3], F32, tag="sm")
            for hc in range(len(ACH)):
                nc.tensor.matmul(
                    LG[:], ASB[:, hc, :], CLSW[:, hc, :],
                    start=(hc == 0), stop=(hc == len(ACH) - 1))
            OSB = stats.tile([BC, 3], F32, tag="osb")
            nc.vector.tensor_scalar(OSB[:], LG[:], RECIP[:], None, ALU.mult)
            nc.vector.tensor_add(OSB[:], OSB[:], CLSB[:])
            nc.sync.dma_start(out_ap[:], OSB[:])

        if repeats == 1:
            body()
        elif repeats < 0:   # python-unrolled (TimelineSim-friendly)
            for _ in range(-repeats):
                body()
        else:
            # unroll several bodies per hardware-loop trip: the For_i loop
            # boundary (sem resets + engine resync) costs tens of us on this
            # part, so amortize it across U bodies
            U = 16
            n_u, rem = divmod(repeats, U)
            if n_u > 0:
                with tc.For_i(0, n_u, 1):
                    for _ in range(U):
                        body()
            if rem > 0:
                with tc.For_i(0, rem, 1):
                    body()

    nc.compile()
    return nc


def host_inputs(inputs):
    """Slice/prepare per-core input maps from the full problem inputs.

    Host work is index bookkeeping only: compact row lists packed into the
    SWDGE gather-index layout.  All tensor arithmetic happens on device.
    """
    hs12 = np.ascontiguousarray(np.asarray(inputs["hidden_states"])[12])  # [B,L,D]
    ts = np.asarray(inputs["token_starts"]).astype(np.int64)
    m = np.ascontiguousarray(np.asarray(inputs["aspect_in_text_mask"], dtype=np.float32))
    gw = np.ascontiguousarray(np.asarray(inputs["guid_W"], dtype=np.float32)[3])
    gb = np.asarray(inputs["guid_b"], dtype=np.float32)[3]
    ln_g = np.asarray(inputs["ln_g"], dtype=np.float32)
    ln_b = np.asarray(inputs["ln_b"], dtype=np.float32)
    cls_W = np.asarray(inputs["cls_W"], dtype=np.float32)
    cls_b = np.asarray(inputs["cls_b"], dtype=np.float32)

    clsw_eff = (ln_g[:, None] * cls_W).astype(np.float32)
    clsw_pad = np.zeros((640, 3), np.float32)
    clsw_pad[:H] = clsw_eff
    clsw_pad[H] = -clsw_eff.sum(0, dtype=np.float32)  # mu-correction row
    clsb_eff = (ln_b @ cls_W + cls_b).astype(np.float32)
    clsb_rep = np.tile(clsb_eff[None, :], (BC, 1)).astype(np.float32)
    iota = np.tile(np.arange(KC, dtype=np.float32)[None, :], (128, 2)).reshape(128, 2 * KC)[:, :KC * 2]
    iota = np.tile(np.concatenate([np.arange(KC, dtype=np.float32)] * 2)[None, :], (128, 1))
    eye = np.eye(128, dtype=np.float32)
    onesrow = np.ones((1, 128), np.float32)

    # compact row lists (index bookkeeping), packed for the fused SWDGE
    # gather: one launch of 8*128 indices into the flattened [B*L, D] view.
    # Sample s occupies gather slots [s*128, (s+1)*128) -> dst chunk s//2,
    # partitions (s%2)*64..  (64 real rows + 64 duplicate pads per sample...
    # actually 64 slots per sample: pair P = chunk P with A in partitions
    # 0:64 and B in 64:128).  The Q7 gather reads idx slot i from
    # [16 + i%16, i//16] on this backend (probed); both 16-partition blocks
    # are written so either read window sees the same values.
    idx_all = np.zeros((B // BC, 128, BC * KC // 16), np.int16)
    tsc_all = np.zeros((B, L), np.float32)
    for b in range(B):
        used = np.unique(ts[b][m[b] > 0])
        assert len(used) <= KC, f"sample {b}: {len(used)} unique rows > {KC}"
        rows = np.full(KC, used[0], np.int64)   # duplicate-pad: always valid
        rows[:len(used)] = used
        core, sl = divmod(b, BC)
        gbase = sl * KC                          # gather slot base for sample
        grows = rows + (sl % BC) * L             # flattened row index
        for i in range(KC):
            g = gbase + i
            idx_all[core, g % 16, g // 16] = grows[i]
            idx_all[core, 16 + g % 16, g // 16] = grows[i]
        lut = {int(v): j for j, v in enumerate(used)}
        for i in range(L):
            tsc_all[b, i] = lut.get(int(ts[b, i]), 0) if m[b, i] > 0 else 0
    in_maps = []
    for c in range(N_CORES):
        sl = slice(c * BC, (c + 1) * BC)
        idx_core = idx_all[c]
        in_maps.append(dict(
            hs=np.ascontiguousarray(hs12[sl]),
            idx=np.ascontiguousarray(idx_core),
            gw=gw,
            gbrow=gb[None, :],
            onesrow=onesrow,
            eye=eye,
            tscT=np.ascontiguousarray(tsc_all[sl].T),
            mT=np.ascontiguousarray(m[sl].T),
            mnat=np.ascontiguousarray(m[sl]),
            iota=iota,
            clsw=clsw_pad,
            clsb=clsb_rep,
        ))
    return in_maps


_PROGRAM = None


def kernel(**inputs):
    global _PROGRAM
    if _PROGRAM is None:
        _PROGRAM = build_program(repeats=1)
    nc = _PROGRAM
    in_maps = host_inputs(inputs)
    res = run_bass_kernel_spmd(nc, in_maps, list(range(N_CORES)), trace=False)
    out = np.concatenate([res.results[c]["out"] for c in range(N_CORES)], axis=0)
    return out.astype(np.float32)


# revision 52
# speedup vs baseline: 1.1113x; 1.1113x over previous
"""Trainium2 Bass kernel for nn_BERT4GCN_53884659695997.

Mathematical reduction
----------------------
In the reference, ``feature`` is reassigned to ``LN(guidance)`` at the top of
every loop iteration, so the GCN block's output is never consumed; only the
last BERT layer's branch (index 3 -> hidden_states layer 12, which skips the
GCN block) reaches the output:

    t[b]      = LN(relu(hs[12,b][ts[b]] @ guid_W[3] + guid_b[3])) * ln_g + ln_b
    logits[b] = ((t[b] * m[b,:,None]).sum(0) / m[b].sum(0)) @ cls_W + cls_b

(verified numerically against the jax reference to ~7e-7 rel err).

Row gathers commute with the row-wise ops (matmul-by-row / relu / LN), so the
gather+mask folds into per-source-row weights w[r] = sum_i m[i]*[ts[i]==r].
Only rows with w[r] != 0 can reach the output (at most |unique(ts[m>0])| ~ 51
per sample), so each sample's work is compacted to KC=64 rows, and two
samples are packed per 128-partition tile (pair P = samples 2P, 2P+1), which
halves the per-sample instruction count.  The compact row lists are emitted
host-side (pure index bookkeeping); ONE fused SWDGE gather DMA per iteration
pulls all 8 samples' rows out of the flattened [B*L, D] view (a dma_gather
launch costs ~2.5us on silicon and rows stream at ~111 GB/s, so batching the
launch and minimizing gathered bytes are what the hardware actually rewards).

The gathered rows land row-major [j, d]; the guidance matmul contracts over d,
so six 128-col PE transposes produce the [d, j] stationary operand.  The
guidance matmul streams guid_W as float32r (full rate for moving dims >= 256);
bias enters as a K=1 ones-row matmul.  The emission is software-pipelined in
five stages across three iterations of lag so the PE stream never waits on the
DVE/ACT stats or copy stages, and all activation functions (Relu / Copy /
Abs_reciprocal_sqrt) live in one act-table set so no LoadActFuncSet lands in
the steady-state loop.

LN folds into the reduction: with per-row stats (mu, rs) from one-pass
bn_stats/bn_aggr and w2 = w * rs,

    sum_r w[r]*(GR[r,:]-mu[r])*rs[r] = GR^T @ w2 - (mu . w2) * ones

mu is carried as a 601st column of the activation tile so the aspect matmul
emits the correction term alongside, and cls_W gains a 601st row equal to
-colsum(ln_g*cls_W), which applies the correction exactly.  ln_g / ln_b fold
into cls_W / cls_b host-side (exact fp32 linear algebra).

Sharding: data-parallel over batch B=64 -> 8 samples per core on 8 cores.
"""

import numpy as np
from contextlib import ExitStack

import concourse.bass as bass
import concourse.tile as tile
from concourse import bacc, mybir
from concourse.bass_utils import run_bass_kernel_spmd

F32 = mybir.dt.float32
F32R = mybir.dt.float32r
I16 = mybir.dt.int16
BF16 = mybir.dt.bfloat16
AX = mybir.AxisListType
ALU = mybir.AluOpType
ACTF = mybir.ActivationFunctionType

N_CORES = 8
B = 64
BC = B // N_CORES
L = 256
D = 768
H = 600
KC = 64         # compact row budget per sample (unique masked starts ~51)
NP = 4          # sample pairs per core: pair P = samples (2P, 2P+1) sharing 128 partitions
EPS = 1e-5
KT = D // 128   # 6 k-tiles
NCH = ((0, 344), (344, 600))           # guidance chunks, both >= 256 for f32r
# aspect/classifier chunks over the extended 601-wide feature (600 + mu col)
ACH = ((0, 128), (128, 256), (256, 384), (384, 512), (512, 601))
IDXW = KC // 16  # idx cols per sample (SWDGE wraps indices over 16 partitions)


def build_program(repeats: int = 1):
    nc = bacc.Bacc("TRN2", target_bir_lowering=False, debug=False,
                   num_devices=N_CORES, dynamic_dma_scratch_size=32768)

    dr = {}
    def din(name, shape, dt=F32):
        dr[name] = nc.dram_tensor(name, list(shape), dt, kind="ExternalInput").ap()
    din("hs", (BC, L, D))
    din("idx", (128, BC * KC // 16), I16)
    din("gw", (D, H))
    din("gbrow", (1, H))
    din("onesrow", (1, 128))
    din("eye", (128, 128))
    din("tscT", (L, BC))
    din("mT", (L, BC))
    din("mnat", (BC, L))
    din("iota", (128, 2 * KC))
    din("clsw", (640, 3))         # ln_g-folded cls_W + correction row, padded
    din("clsb", (BC, 3))          # ln_b@cls_W + cls_b, replicated rows
    out_ap = nc.dram_tensor("out", [BC, 3], F32, kind="ExternalOutput").ap()

    with tile.TileContext(nc) as tc, ExitStack() as ctx:
        cpool = ctx.enter_context(tc.tile_pool(name="consts", bufs=1))
        gpool = ctx.enter_context(tc.tile_pool(name="hsc", bufs=3))
        tpool = ctx.enter_context(tc.tile_pool(name="hst", bufs=3))
        grpool = ctx.enter_context(tc.tile_pool(name="gr2", bufs=4))
        spool = ctx.enter_context(tc.tile_pool(name="small", bufs=2))
        stats = ctx.enter_context(tc.tile_pool(name="stats", bufs=1))
        pg_ps = ctx.enter_context(tc.tile_pool(name="pg", bufs=3, space="PSUM"))
        pgu_ps = ctx.enter_context(tc.tile_pool(name="pgu", bufs=3, space="PSUM"))
        sm_ps = ctx.enter_context(tc.tile_pool(name="sm", bufs=1, space="PSUM"))
        asp_ps = ctx.enter_context(tc.tile_pool(name="asp", bufs=1, space="PSUM"))

        # ---- constants (loaded once; idx/eye first so gathers start early) ----
        # NOTE: float32r must never touch a DMA on this backend (transfers
        # are lossy); f32r tiles are produced by on-chip engine copies only.
        IDX = cpool.tile([128, BC * KC // 16], I16, tag="idx")
        nc.sync.dma_start(IDX[:], dr["idx"][:])
        EYE = cpool.tile([128, 128], F32, tag="eye")
        nc.sync.dma_start(EYE[:], dr["eye"][:])
        GW0 = cpool.tile([128, KT, H], F32, tag="gw0")
        gw_r = dr["gw"].rearrange("(k p) n -> p k n", p=128)
        for kt in range(KT):   # split so early gathers interleave on the DMA engines
            nc.sync.dma_start(GW0[:, kt, :], gw_r[:, kt, :])
        GW = cpool.tile([128, KT, H], BF16, tag="gw")
        for kt in range(KT):   # spread the one-time cast over two engines
            if kt % 2 == 0:
                nc.vector.tensor_copy(GW[:, kt, :], GW0[:, kt, :])
            else:
                nc.scalar.copy(GW[:, kt, :], GW0[:, kt, :])
        GBROW0 = cpool.tile([1, H], F32, tag="gbrow0")
        nc.sync.dma_start(GBROW0[:], dr["gbrow"][:])
        GBROW = cpool.tile([1, H], BF16, tag="gbrow")
        nc.vector.tensor_copy(GBROW[:], GBROW0[:])
        ONESR0 = cpool.tile([1, 128], F32, tag="onesrow0")
        nc.sync.dma_start(ONESR0[:], dr["onesrow"][:])
        ONESR = cpool.tile([1, 128], BF16, tag="onesrow")
        nc.vector.tensor_copy(ONESR[:], ONESR0[:])
        IOTA = cpool.tile([128, 2 * KC], F32, tag="iota")
        nc.sync.dma_start(IOTA[:], dr["iota"][:])
        TSC = cpool.tile([128, 2, BC], F32, tag="tsc")
        nc.sync.dma_start(TSC[:], dr["tscT"].rearrange("(t p) s -> p t s", p=128))
        MT = cpool.tile([128, 2, BC], F32, tag="mt")
        nc.sync.dma_start(MT[:], dr["mT"].rearrange("(t p) s -> p t s", p=128))
        MN = cpool.tile([BC, L], F32, tag="mn")
        nc.sync.dma_start(MN[:], dr["mnat"][:])
        CLSW = cpool.tile([128, 5, 3], F32, tag="clsw")
        nc.sync.dma_start(CLSW[:], dr["clsw"].rearrange("(c p) n -> p c n", p=128))
        CLSB = cpool.tile([BC, 3], F32, tag="clsb")
        nc.sync.dma_start(CLSB[:], dr["clsb"][:])

        ONESC = cpool.tile([128, 1], F32, tag="onesc")
        nc.vector.memset(ONESC[:], 1.0)
        EPSB = stats.tile([128, 1], F32, tag="epsb")
        nc.vector.memset(EPSB[:], EPS)
        # dummy activation so the preamble exits with the same act-table set
        # the loop body uses -- keeps LoadActFuncSet out of the repeat loop
        DUM = stats.tile([1, 1], F32, tag="dum")
        nc.scalar.activation(DUM[:], EPSB[0:1, :], ACTF.Abs_reciprocal_sqrt)

        # 1/sum(m) per sample
        SM = stats.tile([BC, 1], F32, tag="sm")
        nc.vector.tensor_reduce(SM[:], MN[:], AX.X, ALU.add)
        RECIP = stats.tile([BC, 1], F32, tag="recip")
        nc.vector.reciprocal(RECIP[:], SM[:])

        def body():
            ASPT = asp_ps.tile([128, 5 * BC], F32, tag="aspt")
            # zero the full tile so untouched partitions (chunk 4 rows > 89)
            # contribute exact zeros to the classifier contraction
            nc.vector.memset(ASPT[:], 0.0)
            state = {}

            hst_state = {}

            # one fused SWDGE gather for all 8 samples: 512 rows, one Q7
            # launch (a launch costs ~2.5us on silicon and rows stream at
            # ~111 GB/s, so batch the launch and minimize gathered bytes;
            # splitting it in halves measured the same or slightly worse)
            HSCF = gpool.tile([128, NP, D], F32, tag="hscf")
            nc.gpsimd.dma_gather(
                HSCF[:], dr["hs"].rearrange("b l d -> (b l) d"), IDX[:],
                BC * KC, BC * KC, D)

            def front_a(p):
                """transpose pair p -> copy-to-SBUF (cast to f32r)."""
                TPa = pg_ps.tile([128, 384], F32, tag="pg")
                TPb = pg_ps.tile([128, 384], F32, tag="pg")
                for k in range(3):
                    nc.tensor.transpose(
                        TPa[:, k * 128:(k + 1) * 128],
                        HSCF[:, p, k * 128:(k + 1) * 128], EYE[:])
                for k in range(3):
                    nc.tensor.transpose(
                        TPb[:, k * 128:(k + 1) * 128],
                        HSCF[:, p, (k + 3) * 128:(k + 4) * 128], EYE[:])
                HST = tpool.tile([128, KT, 128], BF16, tag="hst")
                nc.vector.tensor_copy(HST[:, 0:3, :], TPa[:])
                nc.scalar.copy(HST[:, 3:6, :], TPb[:])
                hst_state[p] = HST

            def front_b(p):
                """guidance matmul -> relu -> one-pass LN stats (pair-wide)."""
                HST = hst_state.pop(p)
                GR2 = grpool.tile([128, 601], F32, tag="gr2")
                for ci, (nlo, nhi) in enumerate(NCH):
                    PG = pgu_ps.tile([128, nhi - nlo], F32, tag="pgu")
                    for kt in range(KT):
                        nc.tensor.matmul(
                            PG[:], HST[:, kt, :], GW[:, kt, nlo:nhi],
                            start=(kt == 0), stop=False)
                    nc.tensor.matmul(
                        PG[:], ONESR[:], GBROW[:, nlo:nhi], start=False, stop=True)
                    nc.scalar.activation(GR2[:, nlo:nhi], PG[:], ACTF.Relu)
                # 600 = 4 equal half-chunks of 150 -> bn_aggr pooling is exact
                BST = spool.tile([128, 12], F32, tag="bst")
                nc.vector.bn_stats(BST[:, 0:6], GR2[:, 0:300])
                nc.vector.bn_stats(BST[:, 6:12], GR2[:, 300:600])
                AGG = spool.tile([128, 2], F32, tag="agg")
                nc.vector.bn_aggr(AGG[:], BST[:])
                state[p] = (GR2, AGG)

            def back_stats(p):
                """rstd chain + mask-fused one-hots for both pair members."""
                GR2, AGG = state[p]
                RS = spool.tile([128, 1], F32, tag="rs")
                nc.scalar.activation(RS[:], AGG[:, 1:2], ACTF.Abs_reciprocal_sqrt,
                                     bias=EPSB[:])
                nc.vector.tensor_copy(GR2[:, 600:601], AGG[:, 0:1])
                SOHW = spool.tile([128, 2, 128], F32, tag="soh")
                for it in range(2):
                    for h in range(2):
                        sx = 2 * p + h
                        nc.vector.tensor_scalar(
                            SOHW[:, it, h * KC:(h + 1) * KC],
                            IOTA[:, :KC], TSC[:, it, sx:sx + 1],
                            MT[:, it, sx:sx + 1], ALU.is_equal, ALU.mult)
                state[p] = (GR2, RS, SOHW)

            def back_wg(p):
                """gather-weight matmuls (PE, early in the stream)."""
                GR2, RS, SOHW = state[p]
                WPS = sm_ps.tile([128, 1], F32, tag="sm")
                for it in range(2):
                    nc.tensor.matmul(
                        WPS[:], SOHW[:, it, :], ONESC[:],
                        start=(it == 0), stop=(it == 1))
                W2 = spool.tile([128, 1], F32, tag="w2")
                nc.vector.tensor_mul(W2[:], WPS[:], RS[:])
                state[p] = (GR2, W2)

            def back_asp(p):
                """aspect columns for both pair members (PE, end of stream)."""
                GR2, W2 = state.pop(p)
                for h in range(2):
                    sx = 2 * p + h
                    lo, hi = h * KC, (h + 1) * KC
                    for hc, (hlo, hhi) in enumerate(ACH):
                        nc.tensor.matmul(
                            ASPT[:hhi - hlo, hc * BC + sx:hc * BC + sx + 1],
                            GR2[lo:hi, hlo:hhi], W2[lo:hi, :])

            # software-pipelined emission with iteration lags so the PE
            # stream never stalls on the DVE/ACT copy or stats stages
            LAG = 3
            for i in range(NP + LAG):
                if i >= LAG:
                    back_wg(i - LAG)
                if i < NP:
                    front_a(i)
                if 2 <= i < NP + 2:
                    back_stats(i - 2)
                if 1 <= i <= NP:
                    front_b(i - 1)
                if i >= LAG:
                    back_asp(i - LAG)

            # -------- classifier --------
            ASB = stats.tile([128, 5, BC], F32, tag="asb")
            nc.scalar.copy(ASB[:], ASPT[:])
            LG = sm_ps.tile([BC, 3], F32, tag="sm")
            for hc in range(len(ACH)):
                nc.tensor.matmul(
                    LG[:], ASB[:, hc, :], CLSW[:, hc, :],
                    start=(hc == 0), stop=(hc == len(ACH) - 1))
            OSB = stats.tile([BC, 3], F32, tag="osb")
            nc.vector.tensor_scalar(OSB[:], LG[:], RECIP[:], None, ALU.mult)
            nc.vector.tensor_add(OSB[:], OSB[:], CLSB[:])
            nc.sync.dma_start(out_ap[:], OSB[:])

        if repeats == 1:
            body()
        elif repeats < 0:   # python-unrolled (TimelineSim-friendly)
            for _ in range(-repeats):
                body()
        else:
            # unroll several bodies per hardware-loop trip: the For_i loop
            # boundary (sem resets + engine resync) costs tens of us on this
            # part, so amortize it across U bodies
            U = 16
            n_u, rem = divmod(repeats, U)
            if n_u > 0:
                with tc.For_i(0, n_u, 1):
                    for _ in range(U):
                        body()
            if rem > 0:
                with tc.For_i(0, rem, 1):
                    body()

    nc.compile()
    return nc


def host_inputs(inputs):
    """Slice/prepare per-core input maps from the full problem inputs.

    Host work is index bookkeeping only: compact row lists packed into the
    SWDGE gather-index layout.  All tensor arithmetic happens on device.
    """
    hs12 = np.ascontiguousarray(np.asarray(inputs["hidden_states"])[12])  # [B,L,D]
    ts = np.asarray(inputs["token_starts"]).astype(np.int64)
    m = np.ascontiguousarray(np.asarray(inputs["aspect_in_text_mask"], dtype=np.float32))
    gw = np.ascontiguousarray(np.asarray(inputs["guid_W"], dtype=np.float32)[3])
    gb = np.asarray(inputs["guid_b"], dtype=np.float32)[3]
    ln_g = np.asarray(inputs["ln_g"], dtype=np.float32)
    ln_b = np.asarray(inputs["ln_b"], dtype=np.float32)
    cls_W = np.asarray(inputs["cls_W"], dtype=np.float32)
    cls_b = np.asarray(inputs["cls_b"], dtype=np.float32)

    clsw_eff = (ln_g[:, None] * cls_W).astype(np.float32)
    clsw_pad = np.zeros((640, 3), np.float32)
    clsw_pad[:H] = clsw_eff
    clsw_pad[H] = -clsw_eff.sum(0, dtype=np.float32)  # mu-correction row
    clsb_eff = (ln_b @ cls_W + cls_b).astype(np.float32)
    clsb_rep = np.tile(clsb_eff[None, :], (BC, 1)).astype(np.float32)
    iota = np.tile(np.arange(KC, dtype=np.float32)[None, :], (128, 2)).reshape(128, 2 * KC)[:, :KC * 2]
    iota = np.tile(np.concatenate([np.arange(KC, dtype=np.float32)] * 2)[None, :], (128, 1))
    eye = np.eye(128, dtype=np.float32)
    onesrow = np.ones((1, 128), np.float32)

    # compact row lists (index bookkeeping), packed for the fused SWDGE
    # gather: one launch of 8*128 indices into the flattened [B*L, D] view.
    # Sample s occupies gather slots [s*128, (s+1)*128) -> dst chunk s//2,
    # partitions (s%2)*64..  (64 real rows + 64 duplicate pads per sample...
    # actually 64 slots per sample: pair P = chunk P with A in partitions
    # 0:64 and B in 64:128).  The Q7 gather reads idx slot i from
    # [16 + i%16, i//16] on this backend (probed); both 16-partition blocks
    # are written so either read window sees the same values.
    idx_all = np.zeros((B // BC, 128, BC * KC // 16), np.int16)
    tsc_all = np.zeros((B, L), np.float32)
    for b in range(B):
        used = np.unique(ts[b][m[b] > 0])
        assert len(used) <= KC, f"sample {b}: {len(used)} unique rows > {KC}"
        rows = np.full(KC, used[0], np.int64)   # duplicate-pad: always valid
        rows[:len(used)] = used
        core, sl = divmod(b, BC)
        gbase = sl * KC                          # gather slot base for sample
        grows = rows + (sl % BC) * L             # flattened row index
        for i in range(KC):
            g = gbase + i
            idx_all[core, g % 16, g // 16] = grows[i]
            idx_all[core, 16 + g % 16, g // 16] = grows[i]
        lut = {int(v): j for j, v in enumerate(used)}
        for i in range(L):
            tsc_all[b, i] = lut.get(int(ts[b, i]), 0) if m[b, i] > 0 else 0
    in_maps = []
    for c in range(N_CORES):
        sl = slice(c * BC, (c + 1) * BC)
        idx_core = idx_all[c]
        in_maps.append(dict(
            hs=np.ascontiguousarray(hs12[sl]),
            idx=np.ascontiguousarray(idx_core),
            gw=gw,
            gbrow=gb[None, :],
            onesrow=onesrow,
            eye=eye,
            tscT=np.ascontiguousarray(tsc_all[sl].T),
            mT=np.ascontiguousarray(m[sl].T),
            mnat=np.ascontiguousarray(m[sl]),
            iota=iota,
            clsw=clsw_pad,
            clsb=clsb_rep,
        ))
    return in_maps


_PROGRAM = None


def kernel(**inputs):
    global _PROGRAM
    if _PROGRAM is None:
        _PROGRAM = build_program(repeats=1)
    nc = _PROGRAM
    in_maps = host_inputs(inputs)
    res = run_bass_kernel_spmd(nc, in_maps, list(range(N_CORES)), trace=False)
    out = np.concatenate([res.results[c]["out"] for c in range(N_CORES)], axis=0)
    return out.astype(np.float32)


# revision 54
# speedup vs baseline: 1.2484x; 1.1234x over previous
"""Trainium2 Bass kernel for nn_BERT4GCN_53884659695997.

Mathematical reduction
----------------------
In the reference, ``feature`` is reassigned to ``LN(guidance)`` at the top of
every loop iteration, so the GCN block's output is never consumed; only the
last BERT layer's branch (index 3 -> hidden_states layer 12, which skips the
GCN block) reaches the output:

    t[b]      = LN(relu(hs[12,b][ts[b]] @ guid_W[3] + guid_b[3])) * ln_g + ln_b
    logits[b] = ((t[b] * m[b,:,None]).sum(0) / m[b].sum(0)) @ cls_W + cls_b

(verified numerically against the jax reference to ~7e-7 rel err).

Row gathers commute with the row-wise ops (matmul-by-row / relu / LN), so the
gather+mask folds into per-source-row weights w[r] = sum_i m[i]*[ts[i]==r].
Only rows with w[r] != 0 can reach the output (at most |unique(ts[m>0])| ~ 51
per sample), so each sample's work is compacted to KC=64 rows, and two
samples are packed per 128-partition tile (pair P = samples 2P, 2P+1), which
halves the per-sample instruction count.  The compact row lists are emitted
host-side (pure index bookkeeping); ONE fused SWDGE gather DMA per iteration
pulls all 8 samples' rows out of the flattened [B*L, D] view (a dma_gather
launch costs ~2.5us on silicon and rows stream at ~111 GB/s, so batching the
launch and minimizing gathered bytes are what the hardware actually rewards).

The gathered rows land row-major [j, d]; the guidance matmul contracts over d,
so six 128-col PE transposes produce the [d, j] stationary operand.  The
guidance matmul runs in bf16 (both operands cast on-chip; guaranteed full-rate
on silicon and half the stationary bytes, ~2.7e-3 output error vs the 2e-2
tolerance); bias enters as a K=1 ones-row matmul.  The emission is software-pipelined in
five stages across three iterations of lag so the PE stream never waits on the
DVE/ACT stats or copy stages, and all activation functions (Relu / Copy /
Abs_reciprocal_sqrt) live in one act-table set so no LoadActFuncSet lands in
the steady-state loop.

LN folds into the reduction: with per-row stats (mu, rs) from one-pass
bn_stats/bn_aggr and w2 = w * rs,

    sum_r w[r]*(GR[r,:]-mu[r])*rs[r] = GR^T @ w2 - (mu . w2) * ones

mu is carried as a 601st column of the activation tile so the aspect matmul
emits the correction term alongside, and cls_W gains a 601st row equal to
-colsum(ln_g*cls_W), which applies the correction exactly.  ln_g / ln_b fold
into cls_W / cls_b host-side (exact fp32 linear algebra).

Sharding: data-parallel over batch B=64 -> 8 samples per core on 8 cores.
"""

import numpy as np
from contextlib import ExitStack

import concourse.bass as bass
import concourse.tile as tile
from concourse import bacc, mybir
from concourse.bass_utils import run_bass_kernel_spmd

F32 = mybir.dt.float32
F32R = mybir.dt.float32r
I16 = mybir.dt.int16
BF16 = mybir.dt.bfloat16
AX = mybir.AxisListType
ALU = mybir.AluOpType
ACTF = mybir.ActivationFunctionType

N_CORES = 8
B = 64
BC = B // N_CORES
L = 256
D = 768
H = 600
KC = 64         # compact row budget per sample (unique masked starts ~51)
NP = 4          # sample pairs per core: pair P = samples (2P, 2P+1) sharing 128 partitions
EPS = 1e-5
KT = D // 128   # 6 k-tiles
NCH = ((0, 344), (344, 600))           # guidance chunks, both >= 256 for f32r
# aspect/classifier chunks over the extended 601-wide feature (600 + mu col)
ACH = ((0, 128), (128, 256), (256, 384), (384, 512), (512, 601))
IDXW = KC // 16  # idx cols per sample (SWDGE wraps indices over 16 partitions)


def build_program(repeats: int = 1):
    nc = bacc.Bacc("TRN2", target_bir_lowering=False, debug=False,
                   num_devices=N_CORES, dynamic_dma_scratch_size=32768)

    dr = {}
    def din(name, shape, dt=F32):
        dr[name] = nc.dram_tensor(name, list(shape), dt, kind="ExternalInput").ap()
    din("hs", (BC, L, D))
    din("idx", (128, BC * KC // 16), I16)
    din("gw", (D, H))
    din("gbrow", (1, H))
    din("onesrow", (1, 128))
    din("eye", (128, 128))
    din("tscT", (L, BC))
    din("mT", (L, BC))
    din("mnat", (BC, L))
    din("iota", (128, 2 * KC))
    din("clsw", (640, 3))         # ln_g-folded cls_W + correction row, padded
    din("clsb", (BC, 3))          # ln_b@cls_W + cls_b, replicated rows
    out_ap = nc.dram_tensor("out", [BC, 3], F32, kind="ExternalOutput").ap()

    with tile.TileContext(nc) as tc, ExitStack() as ctx:
        cpool = ctx.enter_context(tc.tile_pool(name="consts", bufs=1))
        gpool = ctx.enter_context(tc.tile_pool(name="hsc", bufs=3))
        tpool = ctx.enter_context(tc.tile_pool(name="hst", bufs=3))
        grpool = ctx.enter_context(tc.tile_pool(name="gr2", bufs=4))
        spool = ctx.enter_context(tc.tile_pool(name="small", bufs=2))
        stats = ctx.enter_context(tc.tile_pool(name="stats", bufs=1))
        pg_ps = ctx.enter_context(tc.tile_pool(name="pg", bufs=3, space="PSUM"))
        pgu_ps = ctx.enter_context(tc.tile_pool(name="pgu", bufs=3, space="PSUM"))
        sm_ps = ctx.enter_context(tc.tile_pool(name="sm", bufs=1, space="PSUM"))
        asp_ps = ctx.enter_context(tc.tile_pool(name="asp", bufs=1, space="PSUM"))

        # ---- constants (loaded once; idx/eye first so gathers start early) ----
        # NOTE: float32r must never touch a DMA on this backend (transfers
        # are lossy); f32r tiles are produced by on-chip engine copies only.
        IDX = cpool.tile([128, BC * KC // 16], I16, tag="idx")
        nc.sync.dma_start(IDX[:], dr["idx"][:])
        EYE = cpool.tile([128, 128], F32, tag="eye")
        nc.sync.dma_start(EYE[:], dr["eye"][:])
        GW0 = cpool.tile([128, KT, H], F32, tag="gw0")
        gw_r = dr["gw"].rearrange("(k p) n -> p k n", p=128)
        for kt in range(KT):   # split so early gathers interleave on the DMA engines
            nc.sync.dma_start(GW0[:, kt, :], gw_r[:, kt, :])
        GW = cpool.tile([128, KT, H], BF16, tag="gw")
        for kt in range(KT):   # spread the one-time cast over two engines
            if kt % 2 == 0:
                nc.vector.tensor_copy(GW[:, kt, :], GW0[:, kt, :])
            else:
                nc.scalar.copy(GW[:, kt, :], GW0[:, kt, :])
        GBROW0 = cpool.tile([1, H], F32, tag="gbrow0")
        nc.sync.dma_start(GBROW0[:], dr["gbrow"][:])
        GBROW = cpool.tile([1, H], BF16, tag="gbrow")
        nc.vector.tensor_copy(GBROW[:], GBROW0[:])
        ONESR0 = cpool.tile([1, 128], F32, tag="onesrow0")
        nc.sync.dma_start(ONESR0[:], dr["onesrow"][:])
        ONESR = cpool.tile([1, 128], BF16, tag="onesrow")
        nc.vector.tensor_copy(ONESR[:], ONESR0[:])
        IOTA = cpool.tile([128, 2 * KC], F32, tag="iota")
        nc.sync.dma_start(IOTA[:], dr["iota"][:])
        TSC = cpool.tile([128, 2, BC], F32, tag="tsc")
        nc.sync.dma_start(TSC[:], dr["tscT"].rearrange("(t p) s -> p t s", p=128))
        MT = cpool.tile([128, 2, BC], F32, tag="mt")
        nc.sync.dma_start(MT[:], dr["mT"].rearrange("(t p) s -> p t s", p=128))
        MN = cpool.tile([BC, L], F32, tag="mn")
        nc.sync.dma_start(MN[:], dr["mnat"][:])
        CLSW = cpool.tile([128, 5, 3], F32, tag="clsw")
        nc.sync.dma_start(CLSW[:], dr["clsw"].rearrange("(c p) n -> p c n", p=128))
        CLSB = cpool.tile([BC, 3], F32, tag="clsb")
        nc.sync.dma_start(CLSB[:], dr["clsb"][:])

        ONESC = cpool.tile([128, 1], F32, tag="onesc")
        nc.vector.memset(ONESC[:], 1.0)
        EPSB = stats.tile([128, 1], F32, tag="epsb")
        nc.vector.memset(EPSB[:], EPS)
        # dummy activation so the preamble exits with the same act-table set
        # the loop body uses -- keeps LoadActFuncSet out of the repeat loop
        DUM = stats.tile([1, 1], F32, tag="dum")
        nc.scalar.activation(DUM[:], EPSB[0:1, :], ACTF.Abs_reciprocal_sqrt)

        # 1/sum(m) per sample
        SM = stats.tile([BC, 1], F32, tag="sm")
        nc.vector.tensor_reduce(SM[:], MN[:], AX.X, ALU.add)
        RECIP = stats.tile([BC, 1], F32, tag="recip")
        nc.vector.reciprocal(RECIP[:], SM[:])

        def emit_gather():
            # one fused SWDGE gather for all 8 samples: 512 rows, one launch
            H = gpool.tile([128, NP, D], F32, tag="hscf")
            nc.gpsimd.dma_gather(
                H[:], dr["hs"].rearrange("b l d -> (b l) d"), IDX[:],
                BC * KC, BC * KC, D)
            return H

        def body(HSCF=None):
            ASPT = asp_ps.tile([128, 5 * BC], F32, tag="aspt")
            # zero the full tile so untouched partitions (chunk 4 rows > 89)
            # contribute exact zeros to the classifier contraction
            nc.vector.memset(ASPT[:], 0.0)
            state = {}

            hst_state = {}

            if HSCF is None:
                HSCF = emit_gather()

            def front_a(p):
                """transpose pair p -> copy-to-SBUF (cast to f32r)."""
                TPa = pg_ps.tile([128, 384], F32, tag="pg")
                TPb = pg_ps.tile([128, 384], F32, tag="pg")
                for k in range(3):
                    nc.tensor.transpose(
                        TPa[:, k * 128:(k + 1) * 128],
                        HSCF[:, p, k * 128:(k + 1) * 128], EYE[:])
                for k in range(3):
                    nc.tensor.transpose(
                        TPb[:, k * 128:(k + 1) * 128],
                        HSCF[:, p, (k + 3) * 128:(k + 4) * 128], EYE[:])
                HST = tpool.tile([128, KT, 128], BF16, tag="hst")
                nc.vector.tensor_copy(HST[:, 0:3, :], TPa[:])
                nc.scalar.copy(HST[:, 3:6, :], TPb[:])
                hst_state[p] = HST

            def front_b(p):
                """guidance matmul -> relu -> one-pass LN stats (pair-wide)."""
                HST = hst_state.pop(p)
                GR2 = grpool.tile([128, 601], F32, tag="gr2")
                for ci, (nlo, nhi) in enumerate(NCH):
                    PG = pgu_ps.tile([128, nhi - nlo], F32, tag="pgu")
                    for kt in range(KT):
                        nc.tensor.matmul(
                            PG[:], HST[:, kt, :], GW[:, kt, nlo:nhi],
                            start=(kt == 0), stop=False)
                    nc.tensor.matmul(
                        PG[:], ONESR[:], GBROW[:, nlo:nhi], start=False, stop=True)
                    nc.scalar.activation(GR2[:, nlo:nhi], PG[:], ACTF.Relu)
                # 600 = 4 equal half-chunks of 150 -> bn_aggr pooling is exact
                BST = spool.tile([128, 12], F32, tag="bst")
                nc.vector.bn_stats(BST[:, 0:6], GR2[:, 0:300])
                nc.vector.bn_stats(BST[:, 6:12], GR2[:, 300:600])
                AGG = spool.tile([128, 2], F32, tag="agg")
                nc.vector.bn_aggr(AGG[:], BST[:])
                state[p] = (GR2, AGG)

            def back_stats(p):
                """rstd chain + mask-fused one-hots for both pair members."""
                GR2, AGG = state[p]
                RS = spool.tile([128, 1], F32, tag="rs")
                nc.scalar.activation(RS[:], AGG[:, 1:2], ACTF.Abs_reciprocal_sqrt,
                                     bias=EPSB[:])
                nc.vector.tensor_copy(GR2[:, 600:601], AGG[:, 0:1])
                SOHW = spool.tile([128, 2, 128], F32, tag="soh")
                for it in range(2):
                    for h in range(2):
                        sx = 2 * p + h
                        nc.vector.tensor_scalar(
                            SOHW[:, it, h * KC:(h + 1) * KC],
                            IOTA[:, :KC], TSC[:, it, sx:sx + 1],
                            MT[:, it, sx:sx + 1], ALU.is_equal, ALU.mult)
                state[p] = (GR2, RS, SOHW)

            def back_wg(p):
                """gather-weight matmuls (PE, early in the stream)."""
                GR2, RS, SOHW = state[p]
                WPS = sm_ps.tile([128, 1], F32, tag="sm")
                for it in range(2):
                    nc.tensor.matmul(
                        WPS[:], SOHW[:, it, :], ONESC[:],
                        start=(it == 0), stop=(it == 1))
                W2 = spool.tile([128, 1], F32, tag="w2")
                nc.vector.tensor_mul(W2[:], WPS[:], RS[:])
                state[p] = (GR2, W2)

            def back_asp(p):
                """aspect columns for both pair members (PE, end of stream)."""
                GR2, W2 = state.pop(p)
                for h in range(2):
                    sx = 2 * p + h
                    lo, hi = h * KC, (h + 1) * KC
                    for hc, (hlo, hhi) in enumerate(ACH):
                        nc.tensor.matmul(
                            ASPT[:hhi - hlo, hc * BC + sx:hc * BC + sx + 1],
                            GR2[lo:hi, hlo:hhi], W2[lo:hi, :])

            # software-pipelined emission with iteration lags so the PE
            # stream never stalls on the DVE/ACT copy or stats stages
            LAG = 3
            for i in range(NP + LAG):
                if i >= LAG:
                    back_wg(i - LAG)
                if i < NP:
                    front_a(i)
                if 2 <= i < NP + 2:
                    back_stats(i - 2)
                if 1 <= i <= NP:
                    front_b(i - 1)
                if i >= LAG:
                    back_asp(i - LAG)

            # -------- classifier --------
            ASB = stats.tile([128, 5, BC], F32, tag="asb")
            nc.scalar.copy(ASB[:], ASPT[:])
            LG = sm_ps.tile([BC, 3], F32, tag="sm")
            for hc in range(len(ACH)):
                nc.tensor.matmul(
                    LG[:], ASB[:, hc, :], CLSW[:, hc, :],
                    start=(hc == 0), stop=(hc == len(ACH) - 1))
            OSB = stats.tile([BC, 3], F32, tag="osb")
            nc.vector.tensor_scalar(OSB[:], LG[:], RECIP[:], None, ALU.mult)
            nc.vector.tensor_add(OSB[:], OSB[:], CLSB[:])
            nc.sync.dma_start(out_ap[:], OSB[:])

        if repeats == 1:
            body()
        elif repeats < 0:   # python-unrolled (TimelineSim-friendly)
            for _ in range(-repeats):
                body()
        else:
            # unroll several bodies per hardware-loop trip: the For_i loop
            # boundary (sem resets + engine resync) costs tens of us on this
            # part, so amortize it across U bodies
            U = 16
            n_u, rem = divmod(repeats, U)

            def pipelined_trip(n_bodies):
                # prefetch: body u consumes gather u while gather u+1 streams
                g = emit_gather()
                for u in range(n_bodies):
                    g_next = emit_gather() if u < n_bodies - 1 else None
                    body(g)
                    g = g_next

            if n_u > 0:
                with tc.For_i(0, n_u, 1):
                    pipelined_trip(U)
            if rem > 0:
                with tc.For_i(0, rem, 1):
                    pipelined_trip(rem)

    nc.compile()
    return nc


def host_inputs(inputs):
    """Slice/prepare per-core input maps from the full problem inputs.

    Host work is index bookkeeping only: compact row lists packed into the
    SWDGE gather-index layout.  All tensor arithmetic happens on device.
    """
    hs12 = np.ascontiguousarray(np.asarray(inputs["hidden_states"])[12])  # [B,L,D]
    ts = np.asarray(inputs["token_starts"]).astype(np.int64)
    m = np.ascontiguousarray(np.asarray(inputs["aspect_in_text_mask"], dtype=np.float32))
    gw = np.ascontiguousarray(np.asarray(inputs["guid_W"], dtype=np.float32)[3])
    gb = np.asarray(inputs["guid_b"], dtype=np.float32)[3]
    ln_g = np.asarray(inputs["ln_g"], dtype=np.float32)
    ln_b = np.asarray(inputs["ln_b"], dtype=np.float32)
    cls_W = np.asarray(inputs["cls_W"], dtype=np.float32)
    cls_b = np.asarray(inputs["cls_b"], dtype=np.float32)

    clsw_eff = (ln_g[:, None] * cls_W).astype(np.float32)
    clsw_pad = np.zeros((640, 3), np.float32)
    clsw_pad[:H] = clsw_eff
    clsw_pad[H] = -clsw_eff.sum(0, dtype=np.float32)  # mu-correction row
    clsb_eff = (ln_b @ cls_W + cls_b).astype(np.float32)
    clsb_rep = np.tile(clsb_eff[None, :], (BC, 1)).astype(np.float32)
    iota = np.tile(np.arange(KC, dtype=np.float32)[None, :], (128, 2)).reshape(128, 2 * KC)[:, :KC * 2]
    iota = np.tile(np.concatenate([np.arange(KC, dtype=np.float32)] * 2)[None, :], (128, 1))
    eye = np.eye(128, dtype=np.float32)
    onesrow = np.ones((1, 128), np.float32)

    # compact row lists (index bookkeeping), packed for the fused SWDGE
    # gather: one launch of 8*128 indices into the flattened [B*L, D] view.
    # Sample s occupies gather slots [s*128, (s+1)*128) -> dst chunk s//2,
    # partitions (s%2)*64..  (64 real rows + 64 duplicate pads per sample...
    # actually 64 slots per sample: pair P = chunk P with A in partitions
    # 0:64 and B in 64:128).  The Q7 gather reads idx slot i from
    # [16 + i%16, i//16] on this backend (probed); both 16-partition blocks
    # are written so either read window sees the same values.
    idx_all = np.zeros((B // BC, 128, BC * KC // 16), np.int16)
    tsc_all = np.zeros((B, L), np.float32)
    for b in range(B):
        used = np.unique(ts[b][m[b] > 0])
        assert len(used) <= KC, f"sample {b}: {len(used)} unique rows > {KC}"
        rows = np.full(KC, used[0], np.int64)   # duplicate-pad: always valid
        rows[:len(used)] = used
        core, sl = divmod(b, BC)
        gbase = sl * KC                          # gather slot base for sample
        grows = rows + (sl % BC) * L             # flattened row index
        for i in range(KC):
            g = gbase + i
            idx_all[core, g % 16, g // 16] = grows[i]
            idx_all[core, 16 + g % 16, g // 16] = grows[i]
        lut = {int(v): j for j, v in enumerate(used)}
        for i in range(L):
            tsc_all[b, i] = lut.get(int(ts[b, i]), 0) if m[b, i] > 0 else 0
    in_maps = []
    for c in range(N_CORES):
        sl = slice(c * BC, (c + 1) * BC)
        idx_core = idx_all[c]
        in_maps.append(dict(
            hs=np.ascontiguousarray(hs12[sl]),
            idx=np.ascontiguousarray(idx_core),
            gw=gw,
            gbrow=gb[None, :],
            onesrow=onesrow,
            eye=eye,
            tscT=np.ascontiguousarray(tsc_all[sl].T),
            mT=np.ascontiguousarray(m[sl].T),
            mnat=np.ascontiguousarray(m[sl]),
            iota=iota,
            clsw=clsw_pad,
            clsb=clsb_rep,
        ))
    return in_maps


_PROGRAM = None


def kernel(**inputs):
    global _PROGRAM
    if _PROGRAM is None:
        _PROGRAM = build_program(repeats=1)
    nc = _PROGRAM
    in_maps = host_inputs(inputs)
    res = run_bass_kernel_spmd(nc, in_maps, list(range(N_CORES)), trace=False)
    out = np.concatenate([res.results[c]["out"] for c in range(N_CORES)], axis=0)
    return out.astype(np.float32)
